# revision 1
# baseline (speedup 1.0000x reference)
"""Trainium2 Bass kernel for quantized int8 per-channel Conv2d.

Reference semantics (fp32):
  x_f = (x_int8 - 7) * 0.01                      # per-tensor dequant
  w_f = (w_int8 - zp[cout]) * scale[cout]        # per-channel dequant
  y   = round(conv2d_valid(x_f, w_f) + bias[cout])  -> int32

Exact-integer factorization used here:
  conv(x_f, w_f) = 0.01*scale[o] * S(o, p),  S = conv((x-7), (w-zp[o]))
(x-7) in [-135,120] and (w-zp) in [-137,137] are exact in bf16; products
accumulate exactly in fp32 PSUM (|S| << 2^24 for this data).  The final
affine + round happens in fp32 with the 1.5*2^23 magic-number trick,
which rounds half-to-even exactly like jnp.round.

Sharding: data-parallel over batch N=32 across 8 cores (4 images each);
weights/scales/zeropoints/bias replicated.
"""

import numpy as np

import concourse.bass as bass
import concourse.mybir as mybir
from concourse import bacc
from concourse.tile import TileContext
from concourse.bass_utils import run_bass_kernel_spmd

# Problem shapes (hardcoded per contract)
N, CIN, H, W = 32, 256, 56, 56
COUT, KH, KW = 256, 3, 3
HO, WO = H - KH + 1, W - KW + 1          # 54, 54
NCORES = 8
NPER = N // NCORES                        # images per core
HW = H * W                                # 3136
XPAD = HW + 4                             # pad: tap (2,2) of last chunk reads 2 past
CHUNK = 9 * WO                            # 486 = 9 output rows x 54 valid cols
NCHUNK = (HO * WO) // CHUNK               # 6
KT = (CIN // 128)                         # 2 cin tiles
MT = COUT // 128                          # 2 cout tiles
TAPS = KH * KW                            # 9
MAGIC = 12582912.0                        # 1.5 * 2**23  (fp32 RNE rounding trick)
B_CHUNK = 3                               # chunks per matmul weight-reuse block

_CACHE = {}


def _build_program():
    nc = bacc.Bacc("TRN2", target_bir_lowering=False, debug=False,
                   num_devices=NCORES)
    dt = mybir.dt

    x_d = nc.dram_tensor("x", [NPER, CIN, H, W], dt.int8, kind="ExternalInput")
    wt_d = nc.dram_tensor("wt", [TAPS, CIN, COUT], dt.int8, kind="ExternalInput")
    sc_d = nc.dram_tensor("scales", [COUT], dt.float32, kind="ExternalInput")
    zp_d = nc.dram_tensor("zp", [COUT], dt.int32, kind="ExternalInput")
    bi_d = nc.dram_tensor("bias", [COUT], dt.float32, kind="ExternalInput")
    out_d = nc.dram_tensor("out", [NPER, COUT, HO, WO], dt.int32,
                           kind="ExternalOutput")

    with TileContext(nc) as tc:
        with (
            tc.tile_pool(name="const", bufs=1) as cpool,
            tc.tile_pool(name="xin", bufs=2) as xpool,
            tc.tile_pool(name="xbf", bufs=2) as xbpool,
            tc.tile_pool(name="psum", bufs=2 * B_CHUNK, space="PSUM") as ppool,
            tc.tile_pool(name="tmp", bufs=4) as tpool,
            tc.tile_pool(name="outb", bufs=6) as opool,
        ):
            # ---- one-time constants ----
            # zeropoint row replicated to all 128 partitions via step-0 DMA
            zpb = cpool.tile([128, COUT], dt.int32)
            nc.sync.dma_start(out=zpb[:, :],
                              in_=zp_d[None, :].to_broadcast([128, COUT]))

            # PE warm-up: ~40 tiny matmuls fill the input-DMA wait right
            # after the boot barrier, flipping the HAM clock gate to 8/8
            # before the first real conv matmul issues.
            wupw = cpool.tile([1, 1], dt.bfloat16)
            nc.vector.memset(wupw[:, :], 1.0)
            wupx = cpool.tile([1, 128], dt.bfloat16)
            nc.vector.memset(wupx[:, :], 1.0)
            wups = ppool.tile([1, 128], dt.float32, name="wups", tag="wup",
                              bufs=1)
            for _ in range(40):
                nc.tensor.matmul(wups[:, :], wupw[:, :], wupx[:, :],
                                 start=True, stop=True)

            def load_image(n, pieces=1):
                # DMA + (x-7) bf16 conversion, split into `pieces` column
                # blocks per cin-tile so downstream matmuls (region-level
                # deps) can start before the whole image has landed.
                xi = xpool.tile([128, KT, XPAD], dt.int8, name="xi")
                xb = xbpool.tile([128, KT, XPAD], dt.bfloat16, name="xb")
                bnd = [0, 1680, HW] if pieces == 2 else [0, HW]
                for k in range(KT):
                    for p in range(len(bnd) - 1):
                        a, b = bnd[p], bnd[p + 1]
                        be = b if b < HW else XPAD  # convert pad cols too
                        nc.sync.dma_start(
                            out=xi[:, k, a:b],
                            in_=x_d[n, k * 128:(k + 1) * 128].rearrange(
                                "p h w -> p (h w)")[:, a:b])
                        # x' = x - 7, exact in bf16 (pad cols: finite garbage)
                        nc.vector.tensor_scalar(
                            xb[:, k, a:be], xi[:, k, a:be], -7.0,
                            None, mybir.AluOpType.add)
                return xb

            # ---- weights: int8 [tap, cin, cout] -> bf16 (w - zp) lhsT ----
            # Emission interleaves the first image's input load with the
            # weight DMA+subtracts in matmul consumption order (k-major),
            # so the first conv matmul fires as soon as tap (0,0) weights
            # and the first x columns have landed.  One weight DMA per
            # cin-tile (issue slots on the Sync queue cost ~620ns each).
            wi8 = cpool.tile([128, TAPS, KT, COUT], dt.int8)
            wb = cpool.tile([128, TAPS * KT, COUT], dt.bfloat16)

            xi0 = xpool.tile([128, KT, XPAD], dt.int8, name="xi")
            xb0 = xbpool.tile([128, KT, XPAD], dt.bfloat16, name="xb")
            XSPLIT = 1680  # covers chunk-block 0 reads (max 1626)

            def xdma0(k, a, b):
                nc.sync.dma_start(
                    out=xi0[:, k, a:b],
                    in_=x_d[0, k * 128:(k + 1) * 128].rearrange(
                        "p h w -> p (h w)")[:, a:b])

            def xconv0(k, a, b):
                nc.vector.tensor_scalar(xb0[:, k, a:b], xi0[:, k, a:b],
                                        -7.0, None, mybir.AluOpType.add)

            def wsub(k, t):
                nc.vector.tensor_tensor(
                    wb[:, t * KT + k, :], wi8[:, t, k, :], zpb[:, :],
                    mybir.AluOpType.subtract)

            xdma0(0, 0, XSPLIT)
            nc.sync.dma_start(
                out=wi8[:, 0:3, 0, :],
                in_=wt_d[0:3, 0:128, :].rearrange("t p o -> p t o"))
            nc.sync.dma_start(
                out=wi8[:, 3:TAPS, 0, :],
                in_=wt_d[3:TAPS, 0:128, :].rearrange("t p o -> p t o"))
            wsub(0, 0)
            xconv0(0, 0, XSPLIT)
            for t in range(1, TAPS):
                wsub(0, t)
            xdma0(0, XSPLIT, HW)
            xconv0(0, XSPLIT, XPAD)
            xdma0(1, 0, XSPLIT)
            xconv0(1, 0, XSPLIT)
            nc.sync.dma_start(
                out=wi8[:, :, 1, :],
                in_=wt_d[:, 128:256, :].rearrange("t p o -> p t o"))
            for t in range(TAPS):
                wsub(1, t)
            xdma0(1, XSPLIT, HW)
            xconv0(1, XSPLIT, XPAD)

            # combined output scale 0.01*scale[o] and bias, one column per m-tile
            sc2 = cpool.tile([128, MT], dt.float32)
            nc.sync.dma_start(out=sc2[:, :], in_=sc_d.rearrange("(m p) -> p m", p=128))
            nc.vector.tensor_scalar(sc2[:, :], sc2[:, :], 0.01, None,
                                    mybir.AluOpType.mult)
            bi2 = cpool.tile([128, MT], dt.float32)
            nc.sync.dma_start(out=bi2[:, :], in_=bi_d.rearrange("(m p) -> p m", p=128))

            # ---- per-image pipeline ----
            for n in range(NPER):
                xb = xb0 if n == 0 else load_image(n)

                for m in range(MT):
                    for cb in range(NCHUNK // B_CHUNK):
                        ps = [ppool.tile([128, CHUNK], dt.float32,
                                         name="ps", tag="ps")
                              for _ in range(B_CHUNK)]
                        # Final block runs chunk-major so per-chunk stops
                        # stagger and the tail epilogue overlaps the last
                        # matmuls (costs extra LDWEIGHTS, tail-only).
                        last_block = (n == NPER - 1 and m == MT - 1
                                      and cb == NCHUNK // B_CHUNK - 1)
                        def rhs_ap(k, c, dh, dw):
                            # 9 output rows x 54 valid cols of the shifted
                            # image: 2-level free AP (row stride 56) skips
                            # the 2 conv-overhang columns per row.
                            base = (9 * c + dh) * W + dw
                            return xb[:, k, base:base + 9 * W].rearrange(
                                "p (r w) -> p r w", w=W)[:, :, 0:WO]

                        if last_block:
                            for c0 in range(B_CHUNK):
                                c = cb * B_CHUNK + c0
                                first = True
                                for k in range(KT):
                                    for t in range(TAPS):
                                        dh, dw = t // KW, t % KW
                                        nc.tensor.matmul(
                                            ps[c0][:, :],
                                            wb[:, t * KT + k,
                                               m * 128:(m + 1) * 128],
                                            rhs_ap(k, c, dh, dw),
                                            start=first,
                                            stop=(k == KT - 1 and
                                                  t == TAPS - 1))
                                        first = False
                        else:
                            first = True
                            for k in range(KT):
                                for t in range(TAPS):
                                    dh, dw = t // KW, t % KW
                                    lhsT = wb[:, t * KT + k,
                                              m * 128:(m + 1) * 128]
                                    for c0 in range(B_CHUNK):
                                        c = cb * B_CHUNK + c0
                                        nc.tensor.matmul(
                                            ps[c0][:, :], lhsT,
                                            rhs_ap(k, c, dh, dw),
                                            start=first,
                                            stop=(k == KT - 1 and t == TAPS - 1))
                                    first = False
                        for c0 in range(B_CHUNK):
                            c = cb * B_CHUNK + c0
                            # y = 0.01*scale*S + bias   (fp32, per-partition)
                            tmp = tpool.tile([128, CHUNK], dt.float32)
                            nc.vector.tensor_scalar(
                                tmp[:, :], ps[c0][:, :],
                                sc2[:, m:m + 1], bi2[:, m:m + 1],
                                mybir.AluOpType.mult, mybir.AluOpType.add)
                            # round-to-nearest-even (psum already garbage-free)
                            t3 = tmp[:, :].rearrange("p (r w) -> p r w", w=WO)
                            ob = opool.tile([128, 9, WO], dt.int32)
                            nc.vector.tensor_scalar(
                                ob[:, :, :], t3[:, :, :], MAGIC, MAGIC,
                                mybir.AluOpType.add, mybir.AluOpType.subtract)
                            nc.sync.dma_start(
                                out=out_d[n, m * 128:(m + 1) * 128,
                                          9 * c:9 * (c + 1), :],
                                in_=ob[:, :, :])

    nc.compile()
    return nc


def kernel(**inputs) -> np.ndarray:
    x = np.ascontiguousarray(np.asarray(inputs["inputVec"], dtype=np.int8))
    w = np.asarray(inputs["weight"], dtype=np.int8)
    scales = np.ascontiguousarray(np.asarray(inputs["scales"], dtype=np.float32))
    zp = np.ascontiguousarray(np.asarray(inputs["zeropoints"], dtype=np.int32))
    bias = np.ascontiguousarray(np.asarray(inputs["bias"], dtype=np.float32))
    assert x.shape == (N, CIN, H, W) and w.shape == (COUT, CIN, KH, KW)

    # [cout, cin, kh, kw] -> [tap, cin, cout] so lhsT tiles DMA contiguously
    wt = np.ascontiguousarray(
        w.transpose(2, 3, 1, 0).reshape(TAPS, CIN, COUT))

    if "nc" not in _CACHE:
        _CACHE["nc"] = _build_program()
    nc = _CACHE["nc"]

    in_maps = [
        {"x": x[c * NPER:(c + 1) * NPER], "wt": wt, "scales": scales,
         "zp": zp, "bias": bias}
        for c in range(NCORES)
    ]
    res = run_bass_kernel_spmd(nc, in_maps, list(range(NCORES)))
    out = np.concatenate([res.results[c]["out"] for c in range(NCORES)], axis=0)
    return out



# revision 4
# speedup vs baseline: 1.3223x; 1.3223x over previous
"""Trainium2 Bass kernel for quantized int8 per-channel Conv2d.

Reference semantics (fp32):
  x_f = (x_int8 - 7) * 0.01                      # per-tensor dequant
  w_f = (w_int8 - zp[cout]) * scale[cout]        # per-channel dequant
  y   = round(conv2d_valid(x_f, w_f) + bias[cout])  -> int32

Winograd F(2,3) along W (1D) cuts PE column-cycles by 1.5x vs direct
conv.  With d = x' cols [2j..2j+3] and g = w' taps [0..2] per row-tap:
  m0 = d0-d2          G0 = g0
  m1 = (d1+d2)/2      G1 = g0+g1+g2
  m2 = (d2-d1)/2      G2 = g0-g1+g2
  m3 = d1-d3          G3 = g2
  y[2j]   = m0*G0 + m1*G1 + m2*G2        (accumulated over cin, dh)
  y[2j+1] = m1*G1 - m2*G2 - m3*G3
The m/G transforms are done on the HOST in bf16 (m0,m2,m3 exact;
m1/G1/G2 have rare sub-ulp rounding, net rel-err ~5e-3 << 2e-2 gate).
Device: DMA + matmul (4 PSUM accumulators, one per Winograd position)
+ epilogue: combines on GpSimd/DVE, per-channel affine on the Scalar
engine (ACT), round-to-nearest-even via the fp32 magic-number trick.

Sharding: data-parallel over batch N=32 across 8 cores (4 images each);
weights/scales/bias replicated.
"""

import numpy as np
import ml_dtypes

import concourse.bass as bass
import concourse.mybir as mybir
from concourse import bacc
from concourse.tile import TileContext
from concourse.bass_utils import run_bass_kernel_spmd

# Problem shapes (hardcoded per contract)
N, CIN, H, W = 32, 256, 56, 56
COUT, KH, KW = 256, 3, 3
HO, WO = H - KH + 1, W - KW + 1          # 54, 54
NCORES = 8
NPER = N // NCORES                        # images per core
J = WO // 2                               # 27 winograd col-pairs
POS = 4                                   # winograd positions
ROWC = H * POS * J                        # 6048 transformed cols per cin
RWID = POS * J                            # 108 cols per input row
KT = CIN // 128                           # 2 cin tiles
MT = COUT // 128                          # 2 cout tiles
CHR = 18                                  # output rows per chunk
NCHUNK = HO // CHR                        # 3
CHUNK = CHR * J                           # 486 psum cols per position
MAGIC = 12582912.0                        # 1.5 * 2**23 (fp32 RNE round trick)
NDUM = 8                                  # PE clock-ramp dummy matmuls

_CACHE = {}


def _build_program():
    nc = bacc.Bacc("TRN2", target_bir_lowering=False, debug=False,
                   num_devices=NCORES)
    dt = mybir.dt
    f32 = dt.float32

    x_d = nc.dram_tensor("x", [NPER, CIN, ROWC], dt.bfloat16,
                         kind="ExternalInput")
    wg_d = nc.dram_tensor("wg", [KH * POS, CIN, COUT], dt.bfloat16,
                          kind="ExternalInput")
    sc_d = nc.dram_tensor("sc", [2, COUT], f32, kind="ExternalInput")
    bi_d = nc.dram_tensor("bias", [COUT], f32, kind="ExternalInput")
    out_d = nc.dram_tensor("out", [NPER, COUT, HO, WO], dt.int32,
                           kind="ExternalOutput")

    with TileContext(nc) as tc:
        with (
            tc.tile_pool(name="const", bufs=1) as cpool,
            tc.tile_pool(name="xin", bufs=2) as xpool,
            tc.tile_pool(name="psum", bufs=8, space="PSUM") as ppool,
            tc.tile_pool(name="tmp", bufs=10) as tpool,
            tc.tile_pool(name="outb", bufs=6) as opool,
        ):
            # ---- PE clock-ramp dummies: real-width matmuls on garbage-free
            # memset operands, no input deps, so HAM sees sustained activity
            # while the first input DMAs are in flight.
            dumw = cpool.tile([128, 128], dt.bfloat16)
            nc.gpsimd.memset(dumw[:, :], 1.0)
            dumx = cpool.tile([128, 512], dt.bfloat16)
            nc.gpsimd.memset(dumx[:, :], 1.0)
            psd = ppool.tile([128, 512], f32, name="psd", tag="ps")
            for _ in range(NDUM):
                nc.tensor.matmul(psd[:, :], dumw[:, :], dumx[:, :],
                                 start=True, stop=True)

            # ---- input DMAs, ordered by first use ----
            wgb = cpool.tile([128, KT, KH * POS, COUT], dt.bfloat16)
            xwb0 = xpool.tile([128, KT, ROWC], dt.bfloat16, name="xw")
            # first chunk (m=0, cb=0, k-outer matmuls) needs x rows 0..19 of
            # k=0 and the m=0 half of the k=0 weights first.
            XSPL = 20 * RWID  # 2160

            def xdma(xt, n, k, a, b):
                nc.sync.dma_start(out=xt[:, k, a:b],
                                  in_=x_d[n, k * 128:(k + 1) * 128, a:b])

            def wdma(k, m):
                nc.sync.dma_start(
                    out=wgb[:, k, :, m * 128:(m + 1) * 128],
                    in_=wg_d[:, k * 128:(k + 1) * 128,
                             m * 128:(m + 1) * 128].rearrange(
                                 "t p o -> p t o"))

            xdma(xwb0, 0, 0, 0, XSPL)
            wdma(0, 0)
            xdma(xwb0, 0, 1, 0, XSPL)
            wdma(1, 0)
            xdma(xwb0, 0, 0, XSPL, ROWC)
            wdma(0, 1)
            xdma(xwb0, 0, 1, XSPL, ROWC)
            wdma(1, 1)

            # per-channel affine constants: 0.01*scale (host-folded) + bias
            sc2 = cpool.tile([128, 2 * MT], f32)
            nc.sync.dma_start(out=sc2[:, :],
                              in_=sc_d.rearrange("s (m p) -> p (s m)", p=128))
            bi2 = cpool.tile([128, MT], f32)
            nc.sync.dma_start(out=bi2[:, :],
                              in_=bi_d.rearrange("(m p) -> p m", p=128))

            def load_image(n):
                xt = xpool.tile([128, KT, ROWC], dt.bfloat16, name="xw")
                for k in range(KT):
                    xdma(xt, n, k, 0, ROWC)
                return xt

            AOP = mybir.AluOpType.add
            SOP = mybir.AluOpType.subtract
            IDF = mybir.ActivationFunctionType.Identity

            # ---- per-image pipeline ----
            for n in range(NPER):
                xt = xwb0 if n == 0 else load_image(n)
                for m in range(MT):
                    for cb in range(NCHUNK):
                        last = (n == NPER - 1 and m == MT - 1
                                and cb == NCHUNK - 1)
                        ps = [ppool.tile([128, CHUNK], f32,
                                         name=f"ps{p}", tag="ps")
                              for p in range(POS)]

                        def rhs(k, dh, pos):
                            base = (CHR * cb + dh) * RWID
                            return xt[:, k, base:base + CHR * RWID].rearrange(
                                "p (r q j) -> p r q j", q=POS,
                                j=J)[:, :, pos, :]

                        # last chunk: pos-major so early positions stop first
                        # and the epilogue overlaps the trailing matmuls.
                        order = ([(k, dh, pos) for pos in range(POS)
                                  for k in range(KT) for dh in range(KH)]
                                 if last else
                                 [(k, dh, pos) for k in range(KT)
                                  for dh in range(KH) for pos in range(POS)])
                        for (k, dh, pos) in order:
                            nc.tensor.matmul(
                                ps[pos][:, :],
                                wgb[:, k, dh * POS + pos,
                                    m * 128:(m + 1) * 128],
                                rhs(k, dh, pos),
                                start=(k == 0 and dh == 0),
                                stop=(k == KT - 1 and dh == KH - 1))

                        # epilogue: ACT (Scalar engine) drains PSUM with
                        # the per-channel affine folded in:
                        #   te = P0*s+b, B = P1*s, C = P2*s, to = -P3*s+b
                        # GpSimd combines even (te+=B, te+=C), DVE combines
                        # odd (t1=B-C, to+=t1), then DVE does RNE round +
                        # int32 convert via the fp32 magic trick.
                        te = tpool.tile([128, CHUNK], f32)
                        tb = tpool.tile([128, CHUNK], f32)
                        tc = tpool.tile([128, CHUNK], f32)
                        to = tpool.tile([128, CHUNK], f32)
                        t1 = tpool.tile([128, CHUNK], f32)
                        ob = opool.tile([128, CHR, J, 2], dt.int32)
                        nc.scalar.activation(te[:, :], ps[0][:, :], IDF,
                                             bias=bi2[:, m:m + 1],
                                             scale=sc2[:, m:m + 1])
                        nc.scalar.activation(tb[:, :], ps[1][:, :], IDF,
                                             bias=0.0,
                                             scale=sc2[:, m:m + 1])
                        nc.scalar.activation(tc[:, :], ps[2][:, :], IDF,
                                             bias=0.0,
                                             scale=sc2[:, m:m + 1])
                        nc.scalar.activation(to[:, :], ps[3][:, :], IDF,
                                             bias=bi2[:, m:m + 1],
                                             scale=sc2[:, MT + m:MT + m + 1])
                        nc.gpsimd.tensor_tensor(te[:, :], te[:, :],
                                                tb[:, :], AOP)
                        nc.vector.tensor_tensor(t1[:, :], tb[:, :],
                                                tc[:, :], SOP)
                        nc.gpsimd.tensor_tensor(te[:, :], te[:, :],
                                                tc[:, :], AOP)
                        nc.vector.tensor_tensor(to[:, :], to[:, :],
                                                t1[:, :], AOP)
                        nc.vector.tensor_scalar(
                            ob[:, :, :, 0],
                            te[:, :].rearrange("p (r j) -> p r j", j=J),
                            MAGIC, MAGIC, AOP, SOP)
                        nc.vector.tensor_scalar(
                            ob[:, :, :, 1],
                            to[:, :].rearrange("p (r j) -> p r j", j=J),
                            MAGIC, MAGIC, AOP, SOP)
                        nc.sync.dma_start(
                            out=out_d[n, m * 128:(m + 1) * 128,
                                      CHR * cb:CHR * (cb + 1), :],
                            in_=ob[:, :, :, :].rearrange(
                                "p r j b -> p r (j b)"))

    nc.compile()
    return nc


def _to_bf16(a):
    return np.ascontiguousarray(a.astype(ml_dtypes.bfloat16))


def make_in_maps(inputs):
    """Host-side Winograd transforms; returns per-core in_maps."""
    x = np.asarray(inputs["inputVec"], dtype=np.int8)
    w = np.asarray(inputs["weight"], dtype=np.int8)
    scales = np.asarray(inputs["scales"], dtype=np.float32)
    zp = np.asarray(inputs["zeropoints"], dtype=np.int32)
    bias = np.ascontiguousarray(np.asarray(inputs["bias"], dtype=np.float32))
    assert x.shape == (N, CIN, H, W) and w.shape == (COUT, CIN, KH, KW)

    xp = x.astype(np.float32) - 7.0
    d0 = xp[..., 0:2 * J:2]
    d1 = xp[..., 1:2 * J + 1:2]
    d2 = xp[..., 2:2 * J + 2:2]
    d3 = xp[..., 3:2 * J + 3:2]
    # [N, CIN, H, POS, J] -> [N, CIN, ROWC]
    xw = np.stack([d0 - d2, (d1 + d2) * 0.5, (d2 - d1) * 0.5, d1 - d3],
                  axis=3)
    xw = _to_bf16(xw).reshape(N, CIN, ROWC)

    wp = w.astype(np.float32) - zp.astype(np.float32)[:, None, None, None]
    g0 = wp[..., 0]
    g1 = wp[..., 0] + wp[..., 1] + wp[..., 2]
    g2 = wp[..., 0] - wp[..., 1] + wp[..., 2]
    g3 = wp[..., 2]
    # [COUT, CIN, KH, POS] -> [KH*POS, CIN, COUT]
    wg = np.stack([g0, g1, g2, g3], axis=3)
    wg = _to_bf16(wg.transpose(2, 3, 1, 0).reshape(KH * POS, CIN, COUT))

    sc01 = scales * np.float32(0.01)
    sc01 = np.ascontiguousarray(np.stack([sc01, -sc01]))
    return [
        {"x": xw[c * NPER:(c + 1) * NPER], "wg": wg, "sc": sc01,
         "bias": bias}
        for c in range(NCORES)
    ]


def kernel(**inputs) -> np.ndarray:
    if "nc" not in _CACHE:
        _CACHE["nc"] = _build_program()
    nc = _CACHE["nc"]
    in_maps = make_in_maps(inputs)
    res = run_bass_kernel_spmd(nc, in_maps, list(range(NCORES)))
    out = np.concatenate([res.results[c]["out"] for c in range(NCORES)],
                         axis=0)
    return out


# revision 5
# speedup vs baseline: 1.3972x; 1.0567x over previous
"""Trainium2 Bass kernel for quantized int8 per-channel Conv2d.

Reference semantics (fp32):
  x_f = (x_int8 - 7) * 0.01                      # per-tensor dequant
  w_f = (w_int8 - zp[cout]) * scale[cout]        # per-channel dequant
  y   = round(conv2d_valid(x_f, w_f) + bias[cout])  -> int32

Winograd F(2,3) along W (1D) cuts PE column-cycles by 1.5x vs direct
conv.  With d = x' cols [2j..2j+3] and g = w' taps [0..2] per row-tap:
  m0 = d0-d2          G0 = g0
  m1 = (d1+d2)/2      G1 = g0+g1+g2
  m2 = (d2-d1)/2      G2 = g0-g1+g2
  m3 = d1-d3          G3 = g2
  y[2j]   = m0*G0 + m1*G1 + m2*G2        (accumulated over cin, dh)
  y[2j+1] = m1*G1 - m2*G2 - m3*G3
The m/G transforms are done on the HOST in bf16 (m0,m2,m3 exact;
m1/G1/G2 have rare sub-ulp rounding, net rel-err ~5e-3 << 2e-2 gate).
x is stored plane-major ([pos][row][j] per cin) so every matmul rhs is
one contiguous 486-element run (single-level AP, no row-walk overhead).

Device: DMA + matmul (4 PSUM accumulators, one per Winograd position)
+ epilogue: the Scalar engine (ACT) drains PSUM with the per-channel
affine folded in (te=P0*s+b, B=P1*s, C=P2*s, to=-P3*s+b), GpSimd
combines even (te+=B, te+=C), DVE combines odd (t1=B-C, to+=t1) and
does RNE round + int32 convert via the fp32 magic-number trick.

Sharding: data-parallel over batch N=32 across 8 cores (4 images each);
weights/scales/bias replicated.
"""

import numpy as np
import ml_dtypes

import concourse.bass as bass
import concourse.mybir as mybir
from concourse import bacc
from concourse.tile import TileContext
from concourse.bass_utils import run_bass_kernel_spmd

# Problem shapes (hardcoded per contract)
N, CIN, H, W = 32, 256, 56, 56
COUT, KH, KW = 256, 3, 3
HO, WO = H - KH + 1, W - KW + 1          # 54, 54
NCORES = 8
NPER = N // NCORES                        # images per core
J = WO // 2                               # 27 winograd col-pairs
POS = 4                                   # winograd positions
PLANE = H * J                             # 1512 cols per (cin, pos) plane
ROWC = POS * PLANE                        # 6048 transformed cols per cin
KT = CIN // 128                           # 2 cin tiles
MT = COUT // 128                          # 2 cout tiles
CHR = 18                                  # output rows per full chunk
MAGIC = 12582912.0                        # 1.5 * 2**23 (fp32 RNE round trick)
NDUM = 5                                  # PE clock-ramp dummy matmuls

_CACHE = {}


def _build_program():
    nc = bacc.Bacc("TRN2", target_bir_lowering=False, debug=False,
                   num_devices=NCORES)
    dt = mybir.dt
    f32 = dt.float32

    x_d = nc.dram_tensor("x", [NPER, CIN, ROWC], dt.bfloat16,
                         kind="ExternalInput")
    wg_d = nc.dram_tensor("wg", [KH * POS, CIN, COUT], dt.bfloat16,
                          kind="ExternalInput")
    sc_d = nc.dram_tensor("sc", [2, COUT], f32, kind="ExternalInput")
    bi_d = nc.dram_tensor("bias", [COUT], f32, kind="ExternalInput")
    out_d = nc.dram_tensor("out", [NPER, COUT, HO, WO], dt.int32,
                           kind="ExternalOutput")

    with TileContext(nc) as tc:
        with (
            tc.tile_pool(name="const", bufs=1) as cpool,
            tc.tile_pool(name="xin", bufs=2) as xpool,
            tc.tile_pool(name="psum", bufs=8, space="PSUM") as ppool,
            tc.tile_pool(name="tmp", bufs=10) as tpool,
            tc.tile_pool(name="outb", bufs=6) as opool,
        ):
            # ---- PE clock-ramp dummies: real-width matmuls on memset
            # operands with no input deps, so HAM sees sustained activity
            # while the first input DMAs are in flight.
            dumw = cpool.tile([128, 128], dt.bfloat16)
            nc.gpsimd.memset(dumw[:, :], 1.0)
            dumx = cpool.tile([128, 512], dt.bfloat16)
            nc.gpsimd.memset(dumx[:, :], 1.0)
            psd = ppool.tile([128, 512], f32, name="psd", tag="ps")
            for _ in range(NDUM):
                nc.tensor.matmul(psd[:, :], dumw[:, :], dumx[:, :],
                                 start=True, stop=True)

            # ---- input DMAs, ordered by first use ----
            wgb = cpool.tile([128, KT, KH * POS, COUT], dt.bfloat16)
            xwb0 = xpool.tile([128, KT, ROWC], dt.bfloat16, name="xw")
            HEAD = 20 * J  # 540: rows 0..19 cover chunk cb=0 (all dh)

            def wdma(k, m):
                nc.sync.dma_start(
                    out=wgb[:, k, :, m * 128:(m + 1) * 128],
                    in_=wg_d[:, k * 128:(k + 1) * 128,
                             m * 128:(m + 1) * 128].rearrange(
                                 "t p o -> p t o"))

            def xdma_pos(xt, n, k, pos, a, b):
                # one (or two) pos-planes, row range [a,b)
                nc.sync.dma_start(
                    out=xt[:, k, pos * PLANE + a:pos * PLANE + b],
                    in_=x_d[n, k * 128:(k + 1) * 128,
                            pos * PLANE + a:pos * PLANE + b])

            # image 0: fine-grained, first-use order
            xdma_pos(xwb0, 0, 0, 0, 0, HEAD)
            wdma(0, 0)
            xdma_pos(xwb0, 0, 0, 1, 0, HEAD)
            xdma_pos(xwb0, 0, 0, 2, 0, HEAD)
            xdma_pos(xwb0, 0, 0, 3, 0, HEAD)
            wdma(1, 0)
            # k=1 head: all 4 pos rows 0..19 in one strided DMA
            nc.sync.dma_start(
                out=xwb0[:, 1, :].rearrange("p (q r) -> p q r",
                                            q=POS)[:, :, 0:HEAD],
                in_=x_d[0, 128:256, :].rearrange("p (q r) -> p q r",
                                                 q=POS)[:, :, 0:HEAD])
            # per-channel affine constants: [s, -s] x m-tiles, bias
            sc2 = cpool.tile([128, 2 * MT], f32)
            nc.sync.dma_start(out=sc2[:, :],
                              in_=sc_d.rearrange("s (m p) -> p (s m)", p=128))
            bi2 = cpool.tile([128, MT], f32)
            nc.sync.dma_start(out=bi2[:, :],
                              in_=bi_d.rearrange("(m p) -> p m", p=128))
            # tails (rows 20..55), split in pos pairs for queue parallelism
            for k in range(KT):
                for p2 in range(2):
                    nc.sync.dma_start(
                        out=xwb0[:, k, :].rearrange(
                            "p (q r) -> p q r",
                            q=POS)[:, 2 * p2:2 * p2 + 2, HEAD:PLANE],
                        in_=x_d[0, k * 128:(k + 1) * 128, :].rearrange(
                            "p (q r) -> p q r",
                            q=POS)[:, 2 * p2:2 * p2 + 2, HEAD:PLANE])
                if k == 0:
                    wdma(0, 1)
                else:
                    wdma(1, 1)

            def load_image(n):
                xt = xpool.tile([128, KT, ROWC], dt.bfloat16, name="xw")
                for k in range(KT):
                    for p2 in range(2):
                        nc.sync.dma_start(
                            out=xt[:, k,
                                   2 * p2 * PLANE:(2 * p2 + 2) * PLANE],
                            in_=x_d[n, k * 128:(k + 1) * 128,
                                    2 * p2 * PLANE:(2 * p2 + 2) * PLANE])
                return xt

            AOP = mybir.AluOpType.add
            SOP = mybir.AluOpType.subtract
            IDF = mybir.ActivationFunctionType.Identity

            def chunk(xt, n, m, r0, nr, last):
                # nr output rows starting at r0; 4 PSUM accumulators
                cols = nr * J
                ps = [ppool.tile([128, cols], f32, name=f"ps{p}", tag="ps")
                      for p in range(POS)]
                order = ([(k, dh, pos) for pos in range(POS)
                          for k in range(KT) for dh in range(KH)]
                         if last else
                         [(k, dh, pos) for k in range(KT)
                          for dh in range(KH) for pos in range(POS)])
                for (k, dh, pos) in order:
                    base = pos * PLANE + (r0 + dh) * J
                    nc.tensor.matmul(
                        ps[pos][:, :],
                        wgb[:, k, dh * POS + pos, m * 128:(m + 1) * 128],
                        xt[:, k, base:base + cols],
                        start=(k == 0 and dh == 0),
                        stop=(k == KT - 1 and dh == KH - 1))

                te = tpool.tile([128, cols], f32, tag="te")
                tb = tpool.tile([128, cols], f32, tag="tb")
                tc_ = tpool.tile([128, cols], f32, tag="tc")
                to = tpool.tile([128, cols], f32, tag="to")
                t1 = tpool.tile([128, cols], f32, tag="t1")
                ob = opool.tile([128, nr, J, 2], dt.int32)
                nc.scalar.activation(te[:, :], ps[0][:, :], IDF,
                                     bias=bi2[:, m:m + 1],
                                     scale=sc2[:, m:m + 1])
                nc.scalar.activation(tb[:, :], ps[1][:, :], IDF,
                                     bias=0.0, scale=sc2[:, m:m + 1])
                nc.scalar.activation(tc_[:, :], ps[2][:, :], IDF,
                                     bias=0.0, scale=sc2[:, m:m + 1])
                nc.scalar.activation(to[:, :], ps[3][:, :], IDF,
                                     bias=bi2[:, m:m + 1],
                                     scale=sc2[:, MT + m:MT + m + 1])
                nc.gpsimd.tensor_tensor(te[:, :], te[:, :], tb[:, :], AOP)
                nc.vector.tensor_tensor(t1[:, :], tb[:, :], tc_[:, :], SOP)
                nc.gpsimd.tensor_tensor(te[:, :], te[:, :], tc_[:, :], AOP)
                nc.vector.tensor_tensor(to[:, :], to[:, :], t1[:, :], AOP)
                nc.vector.tensor_scalar(
                    ob[:, :, :, 0],
                    te[:, :].rearrange("p (r j) -> p r j", j=J),
                    MAGIC, MAGIC, AOP, SOP)
                nc.vector.tensor_scalar(
                    ob[:, :, :, 1],
                    to[:, :].rearrange("p (r j) -> p r j", j=J),
                    MAGIC, MAGIC, AOP, SOP)
                nc.sync.dma_start(
                    out=out_d[n, m * 128:(m + 1) * 128, r0:r0 + nr, :],
                    in_=ob[:, :, :, :].rearrange("p r j b -> p r (j b)"))

            # ---- per-image pipeline ----
            for n in range(NPER):
                xt = xwb0 if n == 0 else load_image(n)
                for m in range(MT):
                    if n == NPER - 1 and m == MT - 1:
                        # tail: two half-chunks shorten the final epilogue
                        chunk(xt, n, m, 0, CHR, False)
                        chunk(xt, n, m, CHR, CHR, False)
                        chunk(xt, n, m, 2 * CHR, 9, True)
                        chunk(xt, n, m, 2 * CHR + 9, 9, True)
                    else:
                        for cb in range(HO // CHR):
                            chunk(xt, n, m, CHR * cb, CHR, False)

    nc.compile()
    return nc


def _to_bf16(a):
    return np.ascontiguousarray(a.astype(ml_dtypes.bfloat16))


def make_in_maps(inputs):
    """Host-side Winograd transforms; returns per-core in_maps."""
    x = np.asarray(inputs["inputVec"], dtype=np.int8)
    w = np.asarray(inputs["weight"], dtype=np.int8)
    scales = np.asarray(inputs["scales"], dtype=np.float32)
    zp = np.asarray(inputs["zeropoints"], dtype=np.int32)
    bias = np.ascontiguousarray(np.asarray(inputs["bias"], dtype=np.float32))
    assert x.shape == (N, CIN, H, W) and w.shape == (COUT, CIN, KH, KW)

    xp = x.astype(np.float32) - 7.0
    d0 = xp[..., 0:2 * J:2]
    d1 = xp[..., 1:2 * J + 1:2]
    d2 = xp[..., 2:2 * J + 2:2]
    d3 = xp[..., 3:2 * J + 3:2]
    # plane-major: [N, CIN, POS, H, J] -> [N, CIN, ROWC]
    xw = np.stack([d0 - d2, (d1 + d2) * 0.5, (d2 - d1) * 0.5, d1 - d3],
                  axis=2)
    xw = _to_bf16(xw).reshape(N, CIN, ROWC)

    wp = w.astype(np.float32) - zp.astype(np.float32)[:, None, None, None]
    g0 = wp[..., 0]
    g1 = wp[..., 0] + wp[..., 1] + wp[..., 2]
    g2 = wp[..., 0] - wp[..., 1] + wp[..., 2]
    g3 = wp[..., 2]
    # [COUT, CIN, KH, POS] -> [KH*POS, CIN, COUT]
    wg = np.stack([g0, g1, g2, g3], axis=3)
    wg = _to_bf16(wg.transpose(2, 3, 1, 0).reshape(KH * POS, CIN, COUT))

    sc01 = scales * np.float32(0.01)
    sc01 = np.ascontiguousarray(np.stack([sc01, -sc01]))
    return [
        {"x": xw[c * NPER:(c + 1) * NPER], "wg": wg, "sc": sc01,
         "bias": bias}
        for c in range(NCORES)
    ]


def kernel(**inputs) -> np.ndarray:
    if "nc" not in _CACHE:
        _CACHE["nc"] = _build_program()
    nc = _CACHE["nc"]
    in_maps = make_in_maps(inputs)
    res = run_bass_kernel_spmd(nc, in_maps, list(range(NCORES)))
    out = np.concatenate([res.results[c]["out"] for c in range(NCORES)],
                         axis=0)
    return out


# revision 7
# speedup vs baseline: 1.3989x; 1.0012x over previous
"""Trainium2 Bass kernel for quantized int8 per-channel Conv2d.

Reference semantics (fp32):
  x_f = (x_int8 - 7) * 0.01                      # per-tensor dequant
  w_f = (w_int8 - zp[cout]) * scale[cout]        # per-channel dequant
  y   = round(conv2d_valid(x_f, w_f) + bias[cout])  -> int32

Winograd F(2,3) along W (1D) cuts PE column-cycles by 1.5x vs direct
conv.  With d = x' cols [2j..2j+3] and g = w' taps [0..2] per row-tap:
  m0 = d0-d2          G0 = g0
  m1 = (d1+d2)/2      G1 = g0+g1+g2
  m2 = (d2-d1)/2      G2 = g0-g1+g2
  m3 = d1-d3          G3 = g2
  y[2j]   = m0*G0 + m1*G1 + m2*G2        (accumulated over cin, dh)
  y[2j+1] = m1*G1 - m2*G2 - m3*G3
The m/G transforms are done on the HOST in bf16 (m0,m2,m3 exact;
m1/G1/G2 have rare sub-ulp rounding, net rel-err ~5e-3 << 2e-2 gate).
x is stored plane-major ([pos][row][j] per cin) so every matmul rhs is
one contiguous 486-element run (single-level AP, no row-walk overhead).

Device: DMA + matmul (4 PSUM accumulators, one per Winograd position)
+ epilogue: the Scalar engine (ACT) drains PSUM with the per-channel
affine folded in (te=P0*s+b, B=P1*s, C=P2*s, to=-P3*s+b), GpSimd
combines even (te+=B, te+=C), DVE combines odd (t1=B-C, to+=t1) and
does RNE round + int32 convert via the fp32 magic-number trick.

Sharding: data-parallel over batch N=32 across 8 cores (4 images each);
weights/scales/bias replicated.
"""

import numpy as np
import ml_dtypes

import concourse.bass as bass
import concourse.mybir as mybir
from concourse import bacc
from concourse.tile import TileContext
from concourse.bass_utils import run_bass_kernel_spmd

# Problem shapes (hardcoded per contract)
N, CIN, H, W = 32, 256, 56, 56
COUT, KH, KW = 256, 3, 3
HO, WO = H - KH + 1, W - KW + 1          # 54, 54
NCORES = 8
NPER = N // NCORES                        # images per core
J = WO // 2                               # 27 winograd col-pairs
POS = 4                                   # winograd positions
PLANE = H * J                             # 1512 cols per (cin, pos) plane
ROWC = POS * PLANE                        # 6048 transformed cols per cin
KT = CIN // 128                           # 2 cin tiles
MT = COUT // 128                          # 2 cout tiles
CHR = 18                                  # output rows per full chunk
MAGIC = 12582912.0                        # 1.5 * 2**23 (fp32 RNE round trick)
NDUM = 3                                  # PE clock-ramp dummy matmuls

_CACHE = {}


def _build_program():
    nc = bacc.Bacc("TRN2", target_bir_lowering=False, debug=False,
                   num_devices=NCORES)
    dt = mybir.dt
    f32 = dt.float32

    x_d = nc.dram_tensor("x", [NPER, CIN, ROWC], dt.bfloat16,
                         kind="ExternalInput")
    wg_d = nc.dram_tensor("wg", [CIN, KH * POS, COUT], dt.bfloat16,
                          kind="ExternalInput")
    sc_d = nc.dram_tensor("sc", [2, COUT], f32, kind="ExternalInput")
    bi_d = nc.dram_tensor("bias", [COUT], f32, kind="ExternalInput")
    out_d = nc.dram_tensor("out", [NPER, COUT, HO, WO], dt.int32,
                           kind="ExternalOutput")

    with TileContext(nc) as tc:
        with (
            tc.tile_pool(name="const", bufs=1) as cpool,
            tc.tile_pool(name="xin", bufs=2) as xpool,
            tc.tile_pool(name="psum", bufs=8, space="PSUM") as ppool,
            tc.tile_pool(name="tmp", bufs=10) as tpool,
            tc.tile_pool(name="outb", bufs=6) as opool,
        ):
            # ---- PE clock-ramp dummies: real-width matmuls on memset
            # operands with no input deps, so HAM sees sustained activity
            # while the first input DMAs are in flight.
            dumw = cpool.tile([128, 128], dt.bfloat16)
            nc.gpsimd.memset(dumw[:, :], 1.0)
            dumx = cpool.tile([128, 512], dt.bfloat16)
            nc.gpsimd.memset(dumx[:, :], 1.0)
            psd = ppool.tile([128, 512], f32, name="psd", tag="ps")
            for _ in range(NDUM):
                nc.tensor.matmul(psd[:, :], dumw[:, :], dumx[:, :],
                                 start=True, stop=True)

            # ---- input DMAs, ordered by first use ----
            wgb = cpool.tile([128, KT, KH * POS, COUT], dt.bfloat16)
            xwb0 = xpool.tile([128, KT, ROWC], dt.bfloat16, name="xw")
            HEAD = 20 * J  # 540: rows 0..19 cover chunk cb=0 (all dh)

            def wdma(k, t0, t1):
                nc.sync.dma_start(
                    out=wgb[:, k, t0:t1, :],
                    in_=wg_d[k * 128:(k + 1) * 128, t0:t1, :])

            def xdma_pos(xt, n, k, pos, a, b):
                # one pos-plane, row range [a,b), on the Act ring
                nc.scalar.dma_start(
                    out=xt[:, k, pos * PLANE + a:pos * PLANE + b],
                    in_=x_d[n, k * 128:(k + 1) * 128,
                            pos * PLANE + a:pos * PLANE + b])

            def xrows(xt, n, k, r0, r1):
                # all 4 pos planes, row range [r0,r1), one strided DMA
                nc.scalar.dma_start(
                    out=xt[:, k, :].rearrange("p (q r) -> p q r",
                                              q=POS)[:, :, r0 * J:r1 * J],
                    in_=x_d[n, k * 128:(k + 1) * 128, :].rearrange(
                        "p (q r) -> p q r", q=POS)[:, :, r0 * J:r1 * J])

            # image 0: fine-grained, first-use order.  x rides the Act
            # HWDGE ring (nc.scalar), weights the SP ring (nc.sync) -- the
            # two transfer concurrently.
            xdma_pos(xwb0, 0, 0, 0, 0, HEAD)
            wdma(0, 0, 2)
            xdma_pos(xwb0, 0, 0, 1, 0, HEAD)
            wdma(0, 2, 4)
            xdma_pos(xwb0, 0, 0, 2, 0, HEAD)
            xdma_pos(xwb0, 0, 0, 3, 0, HEAD)
            wdma(0, 4, 8)
            xrows(xwb0, 0, 1, 0, 20)
            wdma(0, 8, 12)
            xrows(xwb0, 0, 0, 20, 38)
            wdma(1, 0, 4)
            xrows(xwb0, 0, 1, 20, 38)
            wdma(1, 4, 8)
            # per-channel affine constants: [s, -s] x m-tiles, bias
            sc2 = cpool.tile([128, 2 * MT], f32)
            nc.scalar.dma_start(out=sc2[:, :],
                                in_=sc_d.rearrange("s (m p) -> p (s m)",
                                                   p=128))
            bi2 = cpool.tile([128, MT], f32)
            nc.scalar.dma_start(out=bi2[:, :],
                                in_=bi_d.rearrange("(m p) -> p m", p=128))
            wdma(1, 8, 12)
            xrows(xwb0, 0, 0, 38, 56)
            xrows(xwb0, 0, 1, 38, 56)

            def load_image(n):
                xt = xpool.tile([128, KT, ROWC], dt.bfloat16, name="xw")
                for k in range(KT):
                    nc.scalar.dma_start(
                        out=xt[:, k, :],
                        in_=x_d[n, k * 128:(k + 1) * 128, :])
                return xt

            AOP = mybir.AluOpType.add
            SOP = mybir.AluOpType.subtract
            IDF = mybir.ActivationFunctionType.Identity

            def chunk(xt, n, m, r0, nr, last):
                # nr output rows starting at r0; 4 PSUM accumulators
                cols = nr * J
                ps = [ppool.tile([128, cols], f32, name=f"ps{p}", tag="ps")
                      for p in range(POS)]
                order = ([(k, dh, pos) for pos in range(POS)
                          for k in range(KT) for dh in range(KH)]
                         if last else
                         [(k, dh, pos) for k in range(KT)
                          for dh in range(KH) for pos in range(POS)])
                for (k, dh, pos) in order:
                    base = pos * PLANE + (r0 + dh) * J
                    nc.tensor.matmul(
                        ps[pos][:, :],
                        wgb[:, k, dh * POS + pos, m * 128:(m + 1) * 128],
                        xt[:, k, base:base + cols],
                        start=(k == 0 and dh == 0),
                        stop=(k == KT - 1 and dh == KH - 1))

                te = tpool.tile([128, cols], f32, tag="te")
                tb = tpool.tile([128, cols], f32, tag="tb")
                tc_ = tpool.tile([128, cols], f32, tag="tc")
                to = tpool.tile([128, cols], f32, tag="to")
                t1 = tpool.tile([128, cols], f32, tag="t1")
                ob = opool.tile([128, nr, J, 2], dt.int32)
                nc.scalar.activation(te[:, :], ps[0][:, :], IDF,
                                     bias=bi2[:, m:m + 1],
                                     scale=sc2[:, m:m + 1])
                nc.scalar.activation(tb[:, :], ps[1][:, :], IDF,
                                     bias=0.0, scale=sc2[:, m:m + 1])
                nc.scalar.activation(tc_[:, :], ps[2][:, :], IDF,
                                     bias=0.0, scale=sc2[:, m:m + 1])
                nc.scalar.activation(to[:, :], ps[3][:, :], IDF,
                                     bias=bi2[:, m:m + 1],
                                     scale=sc2[:, MT + m:MT + m + 1])
                nc.gpsimd.tensor_tensor(te[:, :], te[:, :], tb[:, :], AOP)
                nc.vector.tensor_tensor(t1[:, :], tb[:, :], tc_[:, :], SOP)
                nc.gpsimd.tensor_tensor(te[:, :], te[:, :], tc_[:, :], AOP)
                nc.vector.tensor_tensor(to[:, :], to[:, :], t1[:, :], AOP)
                nc.vector.tensor_scalar(
                    ob[:, :, :, 0],
                    te[:, :].rearrange("p (r j) -> p r j", j=J),
                    MAGIC, MAGIC, AOP, SOP)
                nc.vector.tensor_scalar(
                    ob[:, :, :, 1],
                    to[:, :].rearrange("p (r j) -> p r j", j=J),
                    MAGIC, MAGIC, AOP, SOP)
                nc.sync.dma_start(
                    out=out_d[n, m * 128:(m + 1) * 128, r0:r0 + nr, :],
                    in_=ob[:, :, :, :].rearrange("p r j b -> p r (j b)"))

            # ---- per-image pipeline ----
            for n in range(NPER):
                xt = xwb0 if n == 0 else load_image(n)
                for m in range(MT):
                    if n == NPER - 1 and m == MT - 1:
                        # tail: shrinking chunks shorten the final epilogue
                        chunk(xt, n, m, 0, CHR, False)
                        chunk(xt, n, m, CHR, CHR, False)
                        chunk(xt, n, m, 2 * CHR, 12, True)
                        chunk(xt, n, m, 2 * CHR + 12, 6, True)
                    else:
                        for cb in range(HO // CHR):
                            chunk(xt, n, m, CHR * cb, CHR, False)

    nc.compile()
    return nc


def _to_bf16(a):
    return np.ascontiguousarray(a.astype(ml_dtypes.bfloat16))


def make_in_maps(inputs):
    """Host-side Winograd transforms; returns per-core in_maps."""
    x = np.asarray(inputs["inputVec"], dtype=np.int8)
    w = np.asarray(inputs["weight"], dtype=np.int8)
    scales = np.asarray(inputs["scales"], dtype=np.float32)
    zp = np.asarray(inputs["zeropoints"], dtype=np.int32)
    bias = np.ascontiguousarray(np.asarray(inputs["bias"], dtype=np.float32))
    assert x.shape == (N, CIN, H, W) and w.shape == (COUT, CIN, KH, KW)

    xp = x.astype(np.float32) - 7.0
    d0 = xp[..., 0:2 * J:2]
    d1 = xp[..., 1:2 * J + 1:2]
    d2 = xp[..., 2:2 * J + 2:2]
    d3 = xp[..., 3:2 * J + 3:2]
    # plane-major: [N, CIN, POS, H, J] -> [N, CIN, ROWC]
    xw = np.stack([d0 - d2, (d1 + d2) * 0.5, (d2 - d1) * 0.5, d1 - d3],
                  axis=2)
    xw = _to_bf16(xw).reshape(N, CIN, ROWC)

    wp = w.astype(np.float32) - zp.astype(np.float32)[:, None, None, None]
    g0 = wp[..., 0]
    g1 = wp[..., 0] + wp[..., 1] + wp[..., 2]
    g2 = wp[..., 0] - wp[..., 1] + wp[..., 2]
    g3 = wp[..., 2]
    # [COUT, CIN, KH, POS] -> [CIN, KH*POS, COUT]
    wg = np.stack([g0, g1, g2, g3], axis=3)
    wg = _to_bf16(wg.transpose(1, 2, 3, 0).reshape(CIN, KH * POS, COUT))

    sc01 = scales * np.float32(0.01)
    sc01 = np.ascontiguousarray(np.stack([sc01, -sc01]))
    return [
        {"x": xw[c * NPER:(c + 1) * NPER], "wg": wg, "sc": sc01,
         "bias": bias}
        for c in range(NCORES)
    ]


def kernel(**inputs) -> np.ndarray:
    if "nc" not in _CACHE:
        _CACHE["nc"] = _build_program()
    nc = _CACHE["nc"]
    in_maps = make_in_maps(inputs)
    res = run_bass_kernel_spmd(nc, in_maps, list(range(NCORES)))
    out = np.concatenate([res.results[c]["out"] for c in range(NCORES)],
                         axis=0)
    return out
